# revision 12
# baseline (speedup 1.0000x reference)
import sys

sys.path.insert(0, "/opt/trn_rl_repo")

import numpy as np
import ml_dtypes

import bass_rust
import concourse.bass as bass
import concourse.mybir as mybir
import concourse.tile as tile
from concourse.bass import ts, ds
from concourse.bass_utils import run_bass_kernel_spmd
from concourse.vector_clock import ScopedClock

dt = mybir.dt
AF = mybir.ActivationFunctionType
OP = mybir.AluOpType
F8 = ml_dtypes.float8_e4m3
BF16 = ml_dtypes.bfloat16

H = 4
D = 512
MLP = 1024
S = 1024
B = 16
NCORES = 8
BL = B // NCORES
EPS = 1e-6
KD = D // 128
KM = MLP // 128
MT = S // 128
QN = S // 512
INV_SQRT_HD = float(1.0 / np.sqrt(128.0))

SW = 32.0
SX = 16.0
SQK = 64.0
SC = 64.0

_MAX_WAITS = int(__import__('os').environ.get('MAXW', '1'))

PHASE_LOG = []


def _mark(nc, label):
    PHASE_LOG.append((label, int(nc.get_next_instruction_name().split("-")[1])))


class CompatTileContext(tile.TileContext):

    _nop_ctr = 0

    def _hoist_waits(self, inst):
        si = inst.sync_info
        if si is None:
            return
        waits = list(si.on_wait)
        if len(waits) <= _MAX_WAITS:
            return
        if inst.engine == mybir.EngineType.Unassigned:
            return
        keep = [waits[-1]]
        for w in waits[:-1]:
            CompatTileContext._nop_ctr += 1
            nop = mybir.InstNoOp(
                name=f"waitnop-{CompatTileContext._nop_ctr}", ins=[], outs=[]
            )
            nop.engine = inst.engine
            nop.sync_info = bass_rust.SyncInfo(on_wait=[w], on_update=[])
            super()._add_instruction(nop)
        inst.sync_info = bass_rust.SyncInfo(on_wait=keep, on_update=list(si.on_update))

    def _add_instruction(self, inst):
        self._hoist_waits(inst)
        super()._add_instruction(inst)

    def _drain_and_barrier(self, tick_clock, wait_clock):
        drain_inst = self.nc.sync.drain()
        wait_clock.add_sem_waits(
            drain_inst.ins, ScopedClock({None: tick_clock.global_clock})
        )
        inst = drain_inst.ins
        si = inst.sync_info
        waits = list(si.on_wait)
        if len(waits) > _MAX_WAITS:
            inst.sync_info = bass_rust.SyncInfo(
                on_wait=waits[:_MAX_WAITS], on_update=list(si.on_update)
            )
            for w in waits[_MAX_WAITS:]:
                nop = self.nc.sync.nop()
                nop.ins.sync_info = bass_rust.SyncInfo(on_wait=[w], on_update=[])
        self.nc.all_engine_barrier()
        popped = self.nc._tile_sem_poison_stack.pop()
        assert popped is self._sem_poison
        self.nc.clear_and_free_semaphores(list(self.sems.allocated().values()))
        self.nc.all_engine_barrier()


def _build(repeat=1, **opts):
    nc = bass.Bass("TRN2", target_bir_lowering=False, debug=False, num_devices=NCORES)

    f32 = dt.float32
    bf16 = dt.bfloat16
    f8 = dt.float8e4

    def din(name, shape, d=f32):
        return nc.dram_tensor(name, shape, d, kind="ExternalInput").ap()

    x1 = din("x1", [BL, S, D])
    x2 = din("x2", [BL, S, D])
    wsq = {}
    for nm in ["q1", "k1", "v1", "q2", "k2", "v2", "q12", "k12", "v12", "o"]:
        wsq[nm] = din("W" + nm, [KD, 128, D], f8)
    w1 = din("W1", [KD, 128, MLP], f8)
    w2 = din("W2", [KM, 128, D], f8)
    bq = {nm: din("b" + nm, [D]) for nm in ["q1", "q2", "q12"]}
    bvr = {nm: din("bv" + nm, [D]) for nm in ("1", "2", "12")}
    lng = {nm: din(nm, [D]) for nm in ["ln1_g", "ln1_b", "ln2_g", "ln2_b", "lnf_g", "lnf_b"]}
    ident_d = din("ident_bf16", [128, 128], bf16)
    ones_d = din("ones_q", [128, 128], f8)
    b1row_d = din("b1_row", [1, MLP], f8)
    b2row_d = din("b2_row", [1, D], f8)
    borow_d = din("bo_row", [1, D], f8)
    chalf_d = din("chalf", [1, 512], f8)
    ones8_d = din("ones8", [1, 128], f8)
    out = nc.dram_tensor("out", [BL, S, D], f32, kind="ExternalOutput").ap()

    from contextlib import ExitStack

    with CompatTileContext(nc) as tc, ExitStack() as ctx:
        cst = ctx.enter_context(tc.tile_pool(name="cst", bufs=1))
        wpool = ctx.enter_context(tc.tile_pool(name="wpool", bufs=1))
        t8 = ctx.enter_context(tc.tile_pool(name="t8", bufs=opts.get("t8", 8)))
        tqk = ctx.enter_context(tc.tile_pool(name="tqk", bufs=opts.get("tqk", 4)))
        vpool = ctx.enter_context(tc.tile_pool(name="vpool", bufs=2))
        xsb_p = ctx.enter_context(tc.tile_pool(name="xsb", bufs=1))
        xrow = ctx.enter_context(tc.tile_pool(name="xrow", bufs=opts.get("xrow", 8)))
        xnrow = ctx.enter_context(tc.tile_pool(name="xnrow", bufs=2))
        obuf = ctx.enter_context(tc.tile_pool(name="obuf", bufs=2))
        gpool = ctx.enter_context(tc.tile_pool(name="gpool", bufs=2))
        probs = ctx.enter_context(tc.tile_pool(name="probs", bufs=opts.get("probs_bufs", 3)))
        rrep = ctx.enter_context(tc.tile_pool(name="rrep", bufs=2))
        stp = ctx.enter_context(tc.tile_pool(name="stp", bufs=4))
        pp = ctx.enter_context(tc.tile_pool(name="pp", bufs=opts.get("pp", 2), space="PSUM"))
        ps_sc = ctx.enter_context(tc.tile_pool(name="ps_sc", bufs=opts.get("ps_sc", 2), space="PSUM"))
        ps_ctx = ctx.enter_context(tc.tile_pool(name="ps_ctx", bufs=opts.get("ps_ctx", 1), space="PSUM"))
        ps_den = ctx.enter_context(tc.tile_pool(name="ps_den", bufs=opts.get("ps_den", 1), space="PSUM"))

        ident = cst.tile([128, 128], bf16, tag="ident")
        nc.sync.dma_start(ident[:], ident_d)
        ones_q = cst.tile([128, 128], f8, tag="ones")
        nc.sync.dma_start(ones_q[:], ones_d)
        eps_t = cst.tile([128, 1], f32, tag="eps")
        nc.vector.memset(eps_t[:], EPS)
        b1row = cst.tile([1, MLP], f8, tag="b1row")
        nc.sync.dma_start(b1row[:], b1row_d)
        b2row = cst.tile([1, D], f8, tag="b2row")
        nc.sync.dma_start(b2row[:], b2row_d)
        borow = cst.tile([1, D], f8, tag="borow")
        nc.sync.dma_start(borow[:], borow_d)
        chalf = cst.tile([1, 512], f8, tag="chalf")
        nc.sync.dma_start(chalf[:], chalf_d)
        ones8 = cst.tile([1, 128], f8, tag="ones8")
        nc.sync.dma_start(ones8[:], ones8_d)

        def col_t(name_ap, n, tag):
            t = cst.tile([128, n], f32, tag=tag)
            nc.sync.dma_start(t[:], name_ap.rearrange("(t p) -> p t", p=128))
            return t

        lnT = {nm: col_t(lng[nm], KD, "lnT_" + nm) for nm in lng}
        bqT = {nm: col_t(bq[nm], KD, "bqT_" + nm) for nm in bq}

        def rep_t(src_ap, tag):
            t = cst.tile([128, D], f32, tag=tag)
            bc = bass.AP(tensor=src_ap.tensor, offset=src_ap.offset, ap=[[0, 128], [1, D]])
            nc.sync.dma_start(t[:], bc)
            return t

        bv_rep = {nm: rep_t(bvr[nm], "bvr" + nm) for nm in bvr}

        def load_w(wap, n, tag):
            tiles = []
            for k in range(wap.shape[0]):
                w = wpool.tile([128, n], f8, tag=f"{tag}{k}")
                nc.sync.dma_start(w[:], wap[k])
                tiles.append(w)
            return tiles

        wp = {nm: load_w(wsq[nm], D, "w" + nm) for nm in wsq}
        w1p = load_w(w1, MLP, "w1_")
        w2p = load_w(w2, D, "w2_")

        qk_act = opts.get("qk_act", "q")
        gamma_act = opts.get("gamma_act", ())

        def run(*gens):
            gens = [g for g in gens if g is not None]
            while gens:
                alive = []
                for g in gens:
                    try:
                        next(g)
                        alive.append(g)
                    except StopIteration:
                        continue
                gens = alive

        def _transpose_out(xn_ap, gT, bT, outT, mt, on_act):
            for ft in range(KD):
                pt = pp.tile([128, 128], bf16, tag="pp")
                nc.tensor.transpose(pt[:], xn_ap[:, ts(ft, 128)], ident[:])
                if on_act:
                    nc.scalar.activation(
                        outT[:, ft, ts(mt, 128)], pt[:], AF.Identity,
                        bias=bT[:, ft:ft + 1], scale=gT[:, ft:ft + 1],
                    )
                else:
                    nc.vector.tensor_scalar(
                        outT[:, ft, ts(mt, 128)], pt[:],
                        gT[:, ft:ft + 1], bT[:, ft:ft + 1],
                        op0=OP.mult, op1=OP.add,
                    )

        def ln_rows_to_T(get_row, gT, bT, outT, on_act):
            rows = [get_row(mt) for mt in range(MT)]
            yield
            mv = stp.tile([128, MT, 2], f32, tag="mv2")
            for i in range(MT):
                stats = stp.tile([128, 6], f32, tag="st6")
                nc.vector.bn_stats(stats[:], rows[i])
                nc.vector.bn_aggr(mv[:, i, :], stats[:])
                if i % 2 == 1:
                    yield
            ve = stp.tile([128, MT], f32, tag="veps")
            nc.vector.tensor_scalar(
                ve[:], mv[:, :, 1], eps_t[:, 0:1], 1.0 / (SX * SX),
                op0=OP.add, op1=OP.mult,
            )
            y = stp.tile([128, MT], f32, tag="nwy")
            t = stp.tile([128, MT], f32, tag="nwt")
            nc.vector.memset(y[:], 0.7 * SX)
            for _ in range(4):
                nc.vector.tensor_mul(t[:], y[:], y[:])
                nc.vector.tensor_mul(t[:], t[:], ve[:])
                nc.vector.tensor_scalar(
                    t[:], t[:], -0.5, 1.5, op0=OP.mult, op1=OP.add
                )
                nc.vector.tensor_mul(y[:], y[:], t[:])
            yield
            for i in range(MT):
                xn = xnrow.tile([128, D], bf16, tag="xn")
                nc.vector.tensor_scalar(
                    xn[:], rows[i], mv[:, i, 0:1], y[:, i:i + 1],
                    op0=OP.subtract, op1=OP.mult,
                )
                _transpose_out(xn[:], gT, bT, outT, i, on_act)
                yield

        def ln_dram_to_T(src2d, gT, bT, outT, on_act):
            pairs = []
            for tp in range(MT // 2):
                xr = xrow.tile([128, 2, 512], f32, tag="xr")
                nc.sync.dma_start(
                    xr[:],
                    src2d[ds(tp * 256, 256), :].rearrange("(a p) d -> p a d", p=128),
                )
                pairs.append(xr)

            def get_row(mt):
                return pairs[mt // 2][:, mt % 2, :]
            yield from ln_rows_to_T(get_row, gT, bT, outT, on_act)

        def proj_qk(wtiles, bias_col, rhsT, outT, on_act):
            sc = SQK / (SW * SX)
            for m in range(KD):
                for n in range(QN):
                    ps = pp.tile([128, 512], f32, tag="pp")
                    for k in range(KD):
                        nc.tensor.matmul(
                            ps[:], wtiles[k][:, ts(m, 128)],
                            rhsT[:, k, ds(n * 512, 512)],
                            start=(k == 0), stop=(k == KD - 1),
                        )
                    if on_act:
                        nc.scalar.activation(
                            outT[:, m, ds(n * 512, 512)], ps[:], AF.Identity,
                            bias=(0.0 if bias_col is None else bias_col[:, m:m + 1]),
                            scale=sc,
                        )
                    elif bias_col is None:
                        nc.vector.tensor_scalar(
                            outT[:, m, ds(n * 512, 512)], ps[:],
                            sc, None, op0=OP.mult,
                        )
                    else:
                        nc.vector.tensor_scalar(
                            outT[:, m, ds(n * 512, 512)], ps[:],
                            sc, bias_col[:, m:m + 1],
                            op0=OP.mult, op1=OP.add,
                        )
                    yield

        def proj_v(wtiles, bv16_rep, lhsT, outv):
            for mt in range(MT):
                ps = pp.tile([128, 512], f32, tag="pp")
                for k in range(KD):
                    nc.tensor.matmul(
                        ps[:], lhsT[:, k, ts(mt, 128)], wtiles[k][:],
                        start=(k == 0), stop=(k == KD - 1),
                    )
                nc.vector.scalar_tensor_tensor(
                    out=outv[:, mt, :], in0=ps[:], scalar=1.0 / SW,
                    in1=bv16_rep[:], op0=OP.mult, op1=OP.add,
                )
                yield

        def attention(qT, kT, v, ctxT):
            esc = INV_SQRT_HD / (SQK * SQK)
            for h in range(H):
                for qn in range(QN):
                    qsl = ds(qn * 512, 512)
                    ctx_ps = ps_ctx.tile([128, 512], f32, tag="psc")
                    den_ps = ps_den.tile([128, 512], f32, tag="psd")
                    for t in range(MT // 2):
                        sc = ps_sc.tile([128, 1024], f32, tag="pss")
                        nc.tensor.matmul(
                            sc[:, 0:512], kT[:, h, ts(2 * t, 128)], qT[:, h, qsl],
                            start=True, stop=True,
                        )
                        nc.tensor.matmul(
                            sc[:, 512:1024], kT[:, h, ts(2 * t + 1, 128)], qT[:, h, qsl],
                            start=True, stop=True,
                        )
                        pt = probs.tile([128, 2, 512], f8, tag="pb")
                        nc.scalar.activation(
                            pt[:].rearrange("p a b -> p (a b)"), sc[:],
                            AF.Exp, scale=esc,
                        )
                        for i in range(2):
                            kt = 2 * t + i
                            nc.tensor.matmul(
                                ctx_ps[:], v[:, kt, ts(h, 128)], pt[:, i, :],
                                start=(kt == 0), stop=(kt == MT - 1),
                            )
                            nc.tensor.matmul(
                                den_ps[:], ones_q[:], pt[:, i, :],
                                start=(kt == 0), stop=(kt == MT - 1),
                            )
                    rr = rrep.tile([128, 512], f32, tag="rr")
                    nc.vector.reciprocal(rr[:], den_ps[:])
                    nc.vector.tensor_mul(ctxT[:, h, qsl], ctx_ps[:], rr[:])
                    yield

        def att_out_T(ctxT, resT, outT):
            for m in range(KD):
                for n in range(QN):
                    ps = pp.tile([128, 512], f32, tag="pp")
                    for k in range(KD):
                        nc.tensor.matmul(
                            ps[:], wp["o"][k][:, ts(m, 128)],
                            ctxT[:, k, ds(n * 512, 512)],
                            start=(k == 0), stop=(k == KD - 1),
                        )
                    nc.vector.scalar_tensor_tensor(
                        out=outT[:, m, ds(n * 512, 512)], in0=ps[:],
                        scalar=SX / (SW * SC), in1=resT[:, m, ds(n * 512, 512)],
                        op0=OP.mult, op1=OP.add,
                    )
                    yield

        def att_out_N(ctxT, x1_2d, x_sb):
            for mt in range(MT):
                ps = pp.tile([128, 512], f32, tag="pp")
                nc.tensor.matmul(ps[:], ones8[:], borow[:], start=True, stop=False)
                for k in range(KD):
                    nc.tensor.matmul(
                        ps[:], ctxT[:, k, ts(mt, 128)], wp["o"][k][:],
                        start=False, stop=(k == KD - 1),
                    )
                xr = xrow.tile([128, 2, 512], f32, tag="xr")
                nc.sync.dma_start(xr[:, 0, :], x1_2d[ts(mt, 128), :])
                nc.vector.scalar_tensor_tensor(
                    out=x_sb[:, mt, :], in0=ps[:], scalar=1.0 / (SW * SC),
                    in1=xr[:, 0, :], op0=OP.mult, op1=OP.add,
                )
                yield

        stop_after = opts.get("stop_after", "full")

        def _dump_t8(tag_tile, b):
            for mt in range(MT):
                o = obuf.tile([128, D], f32, tag="ob")
                nc.vector.tensor_copy(
                    o[:].rearrange("p (a b) -> p a b", a=KD),
                    tag_tile[:, 0:KD, ts(mt, 128)],
                )
                nc.sync.dma_start(out[b, ts(mt, 128), :], o[:])

        PHASE_LOG.clear()
        batches = [bb for _ in range(repeat) for bb in range(BL)]
        pre = None
        for bi, b in enumerate(batches):
            nb = batches[bi + 1] if bi + 1 < len(batches) else None
            if stop_after != "full":
                nb = None

            if pre is None:
                _mark(nc, f"b{b}:ln1")
                x1nT = t8.tile([128, KD, S], f8, tag="t8")
                run(ln_dram_to_T(x1[b], lnT["ln1_g"], lnT["ln1_b"], x1nT,
                                 "ln1" in gamma_act))
                _mark(nc, f"b{b}:proj1")
                q1T = tqk.tile([128, KD, S], f8, tag="tqk")
                k1T = tqk.tile([128, KD, S], f8, tag="tqk")
                v1 = vpool.tile([128, MT, D], f8, tag="vp")
                x2nT = t8.tile([128, KD, S], f8, tag="t8")
                run(proj_qk(wp["q1"], bqT["q1"], x1nT, q1T, "q" in qk_act),
                    proj_qk(wp["k1"], None, x1nT, k1T, "k" in qk_act),
                    proj_v(wp["v1"], bv_rep["1"], x1nT, v1),
                    ln_dram_to_T(x2[b], lnT["ln2_g"], lnT["ln2_b"], x2nT,
                                 "ln2" in gamma_act))
            else:
                x1nT, q1T, k1T, v1, x2nT = pre

            if stop_after == "proj":
                q2Tp = tqk.tile([128, KD, S], f8, tag="tqk")
                k2Tp = tqk.tile([128, KD, S], f8, tag="tqk")
                v2p = vpool.tile([128, MT, D], f8, tag="vp")
                run(proj_qk(wp["q2"], bqT["q2"], x2nT, q2Tp, "q" in qk_act),
                    proj_qk(wp["k2"], None, x2nT, k2Tp, "k" in qk_act),
                    proj_v(wp["v2"], bv_rep["2"], x2nT, v2p))
                _dump_t8(q2Tp, b)
                pre = None
                continue

            _mark(nc, f"b{b}:attn1")
            ctx1T = t8.tile([128, KD, S], f8, tag="t8")
            q2T = tqk.tile([128, KD, S], f8, tag="tqk")
            k2T = tqk.tile([128, KD, S], f8, tag="tqk")
            v2 = vpool.tile([128, MT, D], f8, tag="vp")
            run(attention(q1T, k1T, v1, ctx1T),
                proj_qk(wp["q2"], bqT["q2"], x2nT, q2T, "q" in qk_act),
                proj_qk(wp["k2"], None, x2nT, k2T, "k" in qk_act),
                proj_v(wp["v2"], bv_rep["2"], x2nT, v2))
            _mark(nc, f"b{b}:ao1")
            src1T = t8.tile([128, KD, S], f8, tag="t8")
            run(att_out_T(ctx1T, x1nT, src1T))

            if stop_after == "attn1":
                _dump_t8(src1T, b)
                pre = None
                continue

            _mark(nc, f"b{b}:attn2")
            ctx2T = t8.tile([128, KD, S], f8, tag="t8")
            q12T = tqk.tile([128, KD, S], f8, tag="tqk")
            run(attention(q2T, k2T, v2, ctx2T),
                proj_qk(wp["q12"], bqT["q12"], src1T, q12T, "q" in qk_act))
            _mark(nc, f"b{b}:ao2")
            src2T = t8.tile([128, KD, S], f8, tag="t8")
            run(att_out_T(ctx2T, x2nT, src2T))

            _mark(nc, f"b{b}:proj12")
            k12T = tqk.tile([128, KD, S], f8, tag="tqk")
            v12 = vpool.tile([128, MT, D], f8, tag="vp")
            run(proj_qk(wp["k12"], None, src2T, k12T, "k" in qk_act),
                proj_v(wp["v12"], bv_rep["12"], src2T, v12))

            _mark(nc, f"b{b}:attn3")
            ctx3T = t8.tile([128, KD, S], f8, tag="t8")
            if nb is not None:
                x1nT_n = t8.tile([128, KD, S], f8, tag="t8")
                g_ln1n = ln_dram_to_T(x1[nb], lnT["ln1_g"], lnT["ln1_b"],
                                      x1nT_n, "ln1" in gamma_act)
            else:
                x1nT_n, g_ln1n = None, None
            run(attention(q12T, k12T, v12, ctx3T), g_ln1n)
            _mark(nc, f"b{b}:ao3")
            x_sb = xsb_p.tile([128, MT, D], f32, tag="xsb")
            run(att_out_N(ctx3T, x1[b], x_sb))

            _mark(nc, f"b{b}:lnf")
            hT = t8.tile([128, KD, S], f8, tag="t8")
            g_lnf = ln_rows_to_T(
                lambda mt: x_sb[:, mt, :], lnT["lnf_g"], lnT["lnf_b"], hT,
                "lnf" in gamma_act,
            )
            if nb is not None:
                q1T_n = tqk.tile([128, KD, S], f8, tag="tqk")
                k1T_n = tqk.tile([128, KD, S], f8, tag="tqk")
                v1_n = vpool.tile([128, MT, D], f8, tag="vp")
                run(g_lnf,
                    proj_qk(wp["q1"], bqT["q1"], x1nT_n, q1T_n, "q" in qk_act),
                    proj_qk(wp["k1"], None, x1nT_n, k1T_n, "k" in qk_act),
                    proj_v(wp["v1"], bv_rep["1"], x1nT_n, v1_n))
            else:
                run(g_lnf)

            _mark(nc, f"b{b}:mlp")

            def g_mlp():
                for chunk in range(QN):
                    g1c = gpool.tile([128, KM, 512], f8, tag="g1c")
                    for j in range(KM // 2):
                        sc = ps_sc.tile([128, 1024], f32, tag="pss")
                        for half in range(2):
                            m = 2 * j + half
                            psl = sc[:, half * 512:(half + 1) * 512]
                            nc.tensor.matmul(
                                psl, b1row[:, ts(m, 128)], chalf[:],
                                start=True, stop=False,
                            )
                            for k in range(KD):
                                nc.tensor.matmul(
                                    psl, w1p[k][:, ts(m, 128)],
                                    hT[:, k, ds(chunk * 512, 512)],
                                    start=False, stop=(k == KD - 1),
                                )
                        nc.scalar.activation(
                            g1c[:, 2 * j:2 * j + 2, :].rearrange("p a b -> p (a b)"),
                            sc[:], AF.Gelu, scale=1.0 / (SW * SX),
                        )
                        yield
                    for mtl in range(4):
                        mt = chunk * 4 + mtl
                        ps = pp.tile([128, 512], f32, tag="pp")
                        nc.tensor.matmul(ps[:], ones8[:], b2row[:],
                                         start=True, stop=False)
                        for k in range(KM):
                            nc.tensor.matmul(
                                ps[:], g1c[:, k, ts(mtl, 128)], w2p[k][:],
                                start=False, stop=(k == KM - 1),
                            )
                        o = obuf.tile([128, D], f32, tag="ob")
                        nc.vector.scalar_tensor_tensor(
                            out=o[:], in0=ps[:], scalar=1.0 / SW,
                            in1=x_sb[:, mt, :], op0=OP.mult, op1=OP.add,
                        )
                        nc.sync.dma_start(out[b, ts(mt, 128), :], o[:])
                        yield

            if nb is not None:
                x2nT_n = t8.tile([128, KD, S], f8, tag="t8")
                run(g_mlp(),
                    ln_dram_to_T(x2[nb], lnT["ln2_g"], lnT["ln2_b"], x2nT_n,
                                 "ln2" in gamma_act))
                pre = (x1nT_n, q1T_n, k1T_n, v1_n, x2nT_n)
            else:
                run(g_mlp())
                pre = None

    return nc


_NC = None


def _pack_w(W, scale=SW):
    K, M = W.shape
    Wp = (np.asarray(W, np.float64) * scale).astype(np.float32)
    return np.ascontiguousarray(Wp.reshape(K // 128, 128, M)).astype(F8)


def make_in_maps(inputs):
    f64 = np.float64
    g = {k: np.asarray(v, f64) for k, v in inputs.items()}
    bo = g["bo"]

    common = {
        "ident_bf16": np.eye(128, dtype=np.float32).astype(BF16),
        "ones_q": np.full((128, 128), 0.25, np.float32).astype(F8),
        "chalf": np.full((1, 512), 0.5, np.float32).astype(F8),
        "ones8": np.full((1, 128), 8.0, np.float32).astype(F8),
    }
    for nm in ["q1", "k1", "v1", "q2", "k2", "v2", "q12", "k12", "v12", "o"]:
        common["W" + nm] = _pack_w(g["W" + nm])
    common["W1"] = _pack_w(g["W1"])
    common["W2"] = _pack_w(g["W2"])

    common["bq1"] = (g["bq1"] * SQK).astype(np.float32)
    common["bq2"] = (g["bq2"] * SQK).astype(np.float32)
    common["bq12"] = ((g["bq12"] + bo @ g["Wq12"]) * SQK).astype(np.float32)
    common["bv1"] = (g["bv1"] * SX).astype(np.float32)
    common["bv2"] = (g["bv2"] * SX).astype(np.float32)
    common["bv12"] = ((g["bv12"] + bo @ g["Wv12"]) * SX).astype(np.float32)
    common["b1_row"] = (g["b1"] * (SW * SX / 0.5)).astype(np.float32).reshape(1, -1).astype(F8)
    common["b2_row"] = (g["b2"] * (SW / 8.0)).astype(np.float32).reshape(1, -1).astype(F8)
    common["bo_row"] = (bo * (SW * SC / 8.0)).astype(np.float32).reshape(1, -1).astype(F8)
    for nm in ["ln1", "ln2", "lnf"]:
        common[nm + "_g"] = g[nm + "_g"].astype(np.float32)
        common[nm + "_b"] = (g[nm + "_b"] * SX).astype(np.float32)

    x1f = np.asarray(inputs["x1"], dtype=np.float32)
    x2f = np.asarray(inputs["x2"], dtype=np.float32)
    in_maps = []
    for c in range(NCORES):
        m = dict(common)
        m["x1"] = np.ascontiguousarray(x1f[c * BL:(c + 1) * BL])
        m["x2"] = np.ascontiguousarray(x2f[c * BL:(c + 1) * BL])
        in_maps.append(m)
    return in_maps


def kernel(**inputs):
    global _NC
    if _NC is None:
        _NC = _build()
    nc = _NC
    in_maps = make_in_maps(inputs)
    res = run_bass_kernel_spmd(nc, in_maps, list(range(NCORES))).results
    return np.concatenate([res[c]["out"] for c in range(NCORES)], axis=0)


if __name__ == "__main__":
    rng = np.random.default_rng(0)
    ins = {
        "x1": rng.standard_normal((B, S, D), dtype=np.float32),
        "x2": rng.standard_normal((B, S, D), dtype=np.float32),
    }
    for nm in ["q1", "k1", "v1", "q2", "k2", "v2", "q12", "k12", "v12", "o"]:
        ins["W" + nm] = rng.standard_normal((D, D), dtype=np.float32) * 0.02
        ins["b" + nm] = np.zeros(D, np.float32)
    ins["W1"] = rng.standard_normal((D, MLP), dtype=np.float32) * 0.02
    ins["b1"] = np.zeros(MLP, np.float32)
    ins["W2"] = rng.standard_normal((MLP, D), dtype=np.float32) * 0.02
    ins["b2"] = np.zeros(D, np.float32)
    for nm in ["ln1_g", "ln2_g", "lnf_g"]:
        ins[nm] = np.ones(D, np.float32)
    for nm in ["ln1_b", "ln2_b", "lnf_b"]:
        ins[nm] = np.zeros(D, np.float32)
    o = kernel(**ins)
    print("out", o.shape, o.dtype, float(np.abs(o).max()))
